# revision 43
# baseline (speedup 1.0000x reference)
"""Fused transformer block (nn_Block_2388001816768) on 8 Trainium2 NeuronCores.

Sharding: (batch, head-half) -> one core. Core c handles batch c//2 and
heads [8o, 8o+8) where o = c%2, over the FULL sequence. Causal attention
is exact: q-chunk qc attends kv tiles 0..4qc+3; diagonal tiles get a
post-exp memset + tril multiply (exp of the masked region is finite
garbage that is overwritten before AV).

Pipeline (per core):
  P1  LN1 over full T (stats + apply on DVE, rstd via DVE
      fast-inverse-sqrt, transposes on PE) -> h^T in SBUF (f32r)
  P2  per head pair: attention (q-chunks 1,3) zip-interleaved with the
      NEXT pair's QKV projections as PE filler; chunk 0 folded in at
      the tail
  P3  Wo for chunks 1,3 -> pairwise ReduceScatter rs(2), rs(3)
  P4  attention chunk 2 + LN2(tiles 4-7) + Wo chunk 0 mid-phase,
      Wo chunk 2 -> rs(0), rs(1)
  P5  FFN W1+GELU+W2 for qc=1, LN2(tiles 0-3), W1+GELU+W2 for qc=0

Scores run in bf16 with both heads of a pair row-packed in the PE array
(tile_position); exp runs on 2-bank [P,1024] PSUM tiles to amortize the
~352-cycle ACT instruction overhead. Softmax normalization is deferred:
ctx/sums are staged to SBUF (frees PSUM fast), reciprocal via the DVE
approx op, partition-broadcast on GpSimd. Only two ACT table sets are
ever loaded (exp, gelu).

Large matmuls run in float32r, fp32 accum. The FFN runs fp8e4 DoubleRow
(2x PE rate): W1*16 / W2*64 are pre-scaled into fp8's normal range
host-side; the inverse scales fold into the GELU input scale and the
output epilogue. LN scale/shift and the 1/sqrt(HD) score scale are
folded into projection weights host-side.
"""

import numpy as np

import concourse.bacc as bacc
import concourse.bass as bass  # noqa: F401
import concourse.mybir as mybir
import concourse.tile as tile
from concourse.bass_utils import run_bass_kernel_spmd
from concourse.masks import make_identity

B, T, D, H = 4, 2048, 1024, 16
HD = D // H  # 64
FF = 4 * D  # 4096
TQ = T // 2  # output rows per core = 1024
P = 128
HL = H // 2  # heads per core = 8
HPL = HL // 2  # head pairs per core = 4

f32 = mybir.dt.float32
f32r = mybir.dt.float32r
i32 = mybir.dt.int32
bf16 = mybir.dt.bfloat16
fp8 = mybir.dt.float8e4
AF = mybir.ActivationFunctionType
ALU = mybir.AluOpType
DR = mybir.MatmulPerfMode.DoubleRow
W1S = 16.0  # host-side weight scale (fp8 range), undone by activation scale
W2S = 64.0
MAGIC = 0x5F3759DF  # fast inverse sqrt seed
EPS = 1e-5

_CACHE = {}
PHASES = 5  # debug: truncate emission after this phase
DBG = 0  # debug dumps

DT = D // P  # 8 d-tiles
NT = T // P  # 16 t-tiles
NQ = TQ // P  # own-half q-tiles = 8
FT = FF // P  # 32 ff-tiles


def _emit_body(nc, tc, sfx, cst, x2d, p2d, dram):
    (xl_d, wq_d, wk_d, wv_d, wo_d, w1_d, w2_d, bo_d, b1f_d, b2_d, out_d,
     lnst_d) = dram
    ident, ident_b, tril, ones16, qkvb = cst

    with (
        tc.tile_pool(name="ctxp" + sfx, bufs=1) as ctxp,
        tc.tile_pool(name="ptq" + sfx, bufs=3) as ptq,
        tc.tile_pool(name="nrm" + sfx, bufs=1) as nrm,
        tc.tile_pool(name="psA" + sfx, bufs=1, space="PSUM") as psA,
        tc.tile_pool(name="psM" + sfx, bufs=1, space="PSUM") as psM,
    ):
        ctxT = ctxp.tile([P, HPL, T], bf16)  # ctx^T head-pair-stacked

        qTs, kTs, vaugs = [], [], []
        for hp in range(HPL):
            qTs.append(ctxp.tile([P, T], bf16, tag=f"qT{hp}", name=f"qT{hp}"))
            kTs.append(ctxp.tile([P, T], bf16, tag=f"kT{hp}", name=f"kT{hp}"))
            vaugs.append((
                ctxp.tile([P, NT, HD + 1], bf16, tag=f"va{hp}e", name=f"va{hp}e"),
                ctxp.tile([P, NT, HD + 1], bf16, tag=f"va{hp}o", name=f"va{hp}o"),
            ))

        # Pre-touch every tag of the early pools so their regions are
        # fully sized before inner pools (hTp/ln1/whead/ffp) stack above
        # them — late growth under a stacked pool deadlocks the allocator.
        def acc_tile(idx):
            return psM.tile([P, 512], f32, tag="acc" + str(idx % 2), name="acc")

        def rsqrt_dve(pool, out, var, n):
            """out[P, n] = 1/sqrt(var + EPS) entirely on DVE.

            Newton for rsqrt seeded with 1/v (reciprocal_approx_fast);
            converges quadratically for v > 1/3 — LN variances here are
            O(1). Six iterations -> ~1e-6 rel err for v in [0.4, 8]."""
            ve = pool.tile([P, 4], f32, tag="rs_ve", name="rs_ve")[:, 0:n]
            nc.vector.tensor_scalar_add(ve, var, EPS)
            y = out
            nc.vector.reciprocal_approx_fast(y, ve)
            t = pool.tile([P, 4], f32, tag="rs_t", name="rs_t")[:, 0:n]
            for _ in range(6):
                nc.vector.tensor_mul(t, y, y)
                nc.vector.tensor_mul(t, t, ve)
                nc.vector.tensor_scalar(
                    out=t, in0=t, scalar1=-0.5, scalar2=1.5,
                    op0=ALU.mult, op1=ALU.add)
                nc.vector.tensor_mul(y, y, t)

        # ---------------- attention ----------------
        norm_pend = []

        def norm_flush():
            """Broadcast + scale for staged chunk norms. Deferred so the
            GpSimd broadcasts never sit behind a collective_compute on
            the shared GpSimd queue (which would stall the DVE muls and
            the whole attention pipeline)."""
            while norm_pend:
                hp, qc, srwe, srwo, rre, rro = norm_pend.pop(0)
                qsl = slice(qc * 512, (qc + 1) * 512)
                bce = nrm.tile([HD, 512], f32, tag="bce", bufs=2)
                bco = nrm.tile([HD, 512], f32, tag="bco", bufs=2)
                nc.gpsimd.partition_broadcast(bce, rre)
                nc.gpsimd.partition_broadcast(bco, rro)
                nc.vector.tensor_mul(
                    ctxT[0:HD, hp, qsl], srwe[0:HD, :], bce)
                tmp = nrm.tile([HD, 512], bf16, tag="tmp", bufs=1)
                nc.vector.tensor_mul(tmp, srwo[0:HD, :], bco)
                nc.sync.dma_start(ctxT[HD:P, hp, qsl], tmp)

        def emit_norm(hp, qc, ctx_e, ctx_o):
            """Stage the whole ctx (frees PSUM fast), take 1/sum at
            partition 0 via the fast approx reciprocal; the broadcast and
            the ctxT scale run later at norm_flush()."""
            if len(norm_pend) >= 2:
                norm_flush()
            srwe = nrm.tile([HD + 1, 512], f32, tag="srwe", bufs=2)
            srwo = nrm.tile([HD + 1, 512], f32, tag="srwo", bufs=2)
            nc.vector.tensor_copy(srwe, ctx_e)
            nc.vector.tensor_copy(srwo, ctx_o)
            rre = nrm.tile([1, 512], f32, tag="rre", bufs=2)
            rro = nrm.tile([1, 512], f32, tag="rro", bufs=2)
            nc.sync.dma_start(rre, srwe[HD:HD + 1, :])
            nc.sync.dma_start(rro, srwo[HD:HD + 1, :])
            nc.vector.reciprocal_approx_fast(rre, rre)
            nc.vector.reciprocal_approx_fast(rro, rro)
            norm_pend.append((hp, qc, srwe, srwo, rre, rro))

        def attn_groups(hp, qc):
            """List of closures, one per 2-kv-tile group of this chunk.
            The chunk's ctx PSUM tiles are allocated by group 0; the last
            group finishes with the (deferred) softmax normalization."""
            qT, kT = qTs[hp], kTs[hp]
            vaug_e, vaug_o = vaugs[hp]
            qlo = qc * 512
            ng = 2 * (qc + 1)
            state = {}

            def group(n):
                def emit():
                    sps = [psA.tile([P, 1024], f32, tag="sps0", name="sps0"),
                           psA.tile([P, 1024], f32, tag="sps1", name="sps1")]
                    for half in range(2):
                        i = 2 * n + half
                        jd = i - 4 * qc
                        off = max(jd, 0) * P
                        isl = slice(i * P, (i + 1) * P)
                        msl = slice(qlo + off, qlo + 512)
                        osl = slice(half * 512 + off, (half + 1) * 512)
                        nc.tensor.matmul(
                            sps[0][:, osl], kT[0:HD, isl], qT[0:HD, msl],
                            start=True, stop=True, tile_position=(0, 0))
                        nc.tensor.matmul(
                            sps[1][:, osl], kT[HD:P, isl], qT[HD:P, msl],
                            start=True, stop=True, tile_position=(HD, 0))
                    pts = []
                    for hh in range(2):
                        pt = ptq.tile([P, 1024], bf16, tag=f"pt{hh}", name=f"pt{hh}")
                        nc.scalar.activation(pt, sps[hh], AF.Exp)
                        pts.append(pt)
                    for half in range(2):
                        i = 2 * n + half
                        jd = i - 4 * qc
                        if jd >= 0:
                            for pt in pts:
                                if jd > 0:
                                    nc.vector.memset(
                                        pt[:, half * 512:
                                           half * 512 + jd * P]
                                        .bitcast(f32), 0.0)
                                dsl = slice(half * 512 + jd * P,
                                            half * 512 + (jd + 1) * P)
                                nc.vector.tensor_mul(
                                    pt[:, dsl], pt[:, dsl], tril)
                    if n == 0:
                        state["ce"] = psA.tile([P, 512], f32,
                                               tag="ctxe", name="ctxe")
                        state["co"] = psA.tile([P, 512], f32,
                                               tag="ctxo", name="ctxo")
                    ctx_e = state["ce"][0:HD + 1, :]
                    ctx_o = state["co"][0:HD + 1, :]
                    for half in range(2):
                        i = 2 * n + half
                        st = (n == 0 and half == 0)
                        sp = (n == ng - 1 and half == 1)
                        nc.tensor.matmul(
                            ctx_e, vaug_e[:, i, :],
                            pts[0][:, half * 512:(half + 1) * 512],
                            start=st, stop=sp)
                        nc.tensor.matmul(
                            ctx_o, vaug_o[:, i, :],
                            pts[1][:, half * 512:(half + 1) * 512],
                            start=st, stop=sp)
                    if n == ng - 1:
                        emit_norm(hp, qc, ctx_e, ctx_o)
                return emit

            return [group(n) for n in range(ng)]

        def run_chunk(hp, qc):
            for g in attn_groups(hp, qc):
                g()

        # ---------------- LN + transpose helper ----------------
        def ln_batch(srcs, dsts, xtag, pool, xbufs=4, hstats=None):
            """LN + transpose for a batch of up to 4 row-tiles [P, D].
            srcs: list of (dma_src_ap, dtype); dsts: (dst_tile, pos).
            hstats: per-tile (nb_ap, rstd_ap) from host-side stats — skips
            on-device stats and runs the apply on ACT instead of DVE."""
            nb_ = len(srcs)
            xts = []
            if hstats is None:
                mvb = pool.tile([P, 4, 2], f32, tag=xtag + "mv")
            for j, (src, dty) in enumerate(srcs):
                x_t = pool.tile([P, D], dty, tag=xtag + "x", bufs=xbufs)
                nc.sync.dma_start(x_t, src)
                if hstats is None:
                    st = pool.tile([P, 2, 6], f32, tag=xtag + "st", bufs=2)
                    nc.vector.bn_stats(st[:, 0, :], x_t[:, 0:512])
                    nc.vector.bn_stats(st[:, 1, :], x_t[:, 512:1024])
                    nc.vector.bn_aggr(mvb[:, j, :], st)
                xts.append(x_t)
            if hstats is None:
                rstdb = pool.tile([P, 4], f32, tag=xtag + "rstd")
                rsqrt_dve(pool, rstdb[:, 0:nb_], mvb[:, 0:nb_, 1], nb_)
                nbb = pool.tile([P, 4], f32, tag=xtag + "nb")
                nc.vector.tensor_mul(
                    nbb[:, 0:nb_], mvb[:, 0:nb_, 0], rstdb[:, 0:nb_])
            for j, x_t in enumerate(xts):
                h_t = pool.tile([P, D], f32r, tag=xtag + "h", bufs=2)
                for dh in range(2):
                    hsl = slice(dh * 512, (dh + 1) * 512)
                    # h = x*rstd - mu*rstd
                    if hstats is not None:
                        nb_ap, rstd_ap = hstats[j]
                        nc.scalar.activation(
                            h_t[:, hsl], x_t[:, hsl], AF.Identity,
                            bias=nb_ap, scale=rstd_ap)
                    else:
                        nc.vector.tensor_scalar(
                            out=h_t[:, hsl], in0=x_t[:, hsl],
                            scalar1=rstdb[:, j:j + 1],
                            scalar2=nbb[:, j:j + 1],
                            op0=ALU.mult, op1=ALU.subtract)
                    tp = acc_tile(dh).bitcast(f32r)
                    for k in range(4):
                        dt = dh * 4 + k
                        nc.tensor.transpose(
                            tp[:, k * P:(k + 1) * P],
                            h_t[:, dt * P:(dt + 1) * P], ident)
                    dst_t, tpos = dsts[j]
                    dst = dst_t[:, dh * 4:dh * 4 + 4,
                                tpos * P:(tpos + 1) * P]
                    src_ap = tp.rearrange("p (a b) -> p a b", a=4)
                    if dh == 0:
                        nc.scalar.copy(dst, src_ap)
                    else:
                        nc.vector.tensor_copy(dst, src_ap)

        def rs(k):
            nc.gpsimd.collective_compute(
                "ReduceScatter", ALU.add,
                replica_groups=[[0, 1], [2, 3], [4, 5], [6, 7]],
                ins=[p2d[k].opt()],
                outs=[x2d[2 * k:2 * k + 2].opt()])

        # ================= P1 + P2 (need h^T) =================
        with tc.tile_pool(name="hTp" + sfx, bufs=1) as hTp:
            hTc = [hTp.tile([P, DT, 512], f32r, tag=f"hT{i}", name=f"hT{i}")
                   for i in range(4)]
            # ----- P1: LN1 over full T -----
            with tc.tile_pool(name="ln1" + sfx, bufs=1) as ln1:
                lnst_sb = ln1.tile([P, NT, 2], f32, tag="lnst")
                nc.sync.dma_start(lnst_sb, lnst_d[:, :, :])
                for g in range(4):
                    srcs = [(xl_d[(4 * g + j) * P:(4 * g + j + 1) * P, :],
                             f32) for j in range(4)]
                    dsts = [(hTc[g], j) for j in range(4)]
                    hst = [(lnst_sb[:, 4 * g + j, 0:1],
                            lnst_sb[:, 4 * g + j, 1:2]) for j in range(4)]
                    ln_batch(srcs, dsts, "l1", ln1, hstats=hst)

            # ----- P2: QKV zip attention chunks 1,3 (+0) -----
            if PHASES < 2:
                return
            with tc.tile_pool(name="whead" + sfx, bufs=1) as whead:
                def wp_dma(hp):
                    wp = whead.tile([P, 3, DT, 2 * HD], f32r, tag="wp",
                                    bufs=2)
                    for wi, w_dram in enumerate((wq_d, wk_d, wv_d)):
                        nc.sync.dma_start(
                            wp[:, wi],
                            w_dram[:, hp * 2 * HD:(hp + 1) * 2 * HD]
                            .rearrange("(dt q) m -> q dt m", q=P))
                    return wp

                def qkv_units(hp, wp):
                    qT, kT = qTs[hp], kTs[hp]
                    vaug_e, vaug_o = vaugs[hp]

                    def unit(wi, c):
                        def emit():
                            if wi == 0 and c == 0:
                                nc.vector.tensor_copy(
                                    vaug_e[:, :, HD:HD + 1],
                                    ones16.unsqueeze(2))
                                nc.vector.tensor_copy(
                                    vaug_o[:, :, HD:HD + 1],
                                    ones16.unsqueeze(2))
                            pp = acc_tile(0)
                            for dt in range(DT):
                                nc.tensor.matmul(
                                    pp, wp[:, wi, dt, :],
                                    hTc[c][:, dt, :],
                                    start=(dt == 0), stop=(dt == DT - 1))
                            csl = slice(c * 512, (c + 1) * 512)
                            bcol = 3 * hp + wi
                            if wi == 0:
                                nc.vector.tensor_scalar_add(
                                    out=qT[:, csl], in0=pp,
                                    scalar1=qkvb[:, bcol:bcol + 1])
                            elif wi == 1:
                                nc.vector.tensor_scalar_add(
                                    out=kT[:, csl], in0=pp,
                                    scalar1=qkvb[:, bcol:bcol + 1])
                            else:
                                vc = nrm.tile([P, 512], bf16, tag="vc",
                                              bufs=1)
                                nc.vector.tensor_scalar_add(
                                    out=vc, in0=pp,
                                    scalar1=qkvb[:, bcol:bcol + 1])
                                vp = acc_tile(1).bitcast(bf16)
                                for k in range(4):
                                    nc.tensor.transpose(
                                        vp[:, k * P:(k + 1) * P],
                                        vc[:, k * P:(k + 1) * P],
                                        ident_b)
                                vpr = vp[:, 0:512].rearrange(
                                    "p (a b) -> p a b", a=4)
                                ksl = slice(4 * c, 4 * c + 4)
                                nc.vector.tensor_copy(
                                    vaug_e[:, ksl, 0:HD],
                                    vpr[:, :, 0:HD])
                                nc.vector.tensor_copy(
                                    vaug_o[:, ksl, 0:HD],
                                    vpr[:, :, HD:P])
                        return emit

                    return [unit(wi, c) for wi in range(3)
                            for c in range(4)]

                # pair 0 QKV straight (nothing to overlap yet)
                wp0 = wp_dma(0)
                for u in qkv_units(0, wp0):
                    u()
                if DBG == 1:  # dump qT pair 0
                    dbg = nrm.tile([P, T], f32, tag="dbg", bufs=1)
                    nc.vector.tensor_copy(dbg, qTs[0])
                    nc.sync.dma_start(out_d[0:P, :], dbg[:, 0:1024])
                    nc.sync.dma_start(out_d[P:2 * P, :], dbg[:, 1024:2048])
                    return

                for hp in range(HPL - 1):
                    # attention chunks 1,3 of pair hp, zipped 1:1 with
                    # the 12 QKV units of pair hp+1
                    wp_n = wp_dma(hp + 1)
                    filler = qkv_units(hp + 1, wp_n)
                    groups = attn_groups(hp, 1) + attn_groups(hp, 3)
                    nf, ng_ = len(filler), len(groups)
                    fi = 0
                    for gi, g in enumerate(groups):
                        g()
                        want = (gi + 1) * nf // ng_
                        while fi < want:
                            filler[fi]()
                            fi += 1
                # pair 3 attention, with whole chunk-0 units as filler
                run_chunk(3, 1)
                run_chunk(0, 0)
                run_chunk(3, 3)
                run_chunk(1, 0)
            run_chunk(2, 0)
            run_chunk(3, 0)
            if DBG == 2:  # dump ctxT pair 0 (chunks 0,1 done... all 4? chunks 2 later)
                dbg = nrm.tile([P, T], f32, tag="dbg", bufs=1)
                nc.vector.tensor_copy(dbg, ctxT[:, 0, :])
                nc.sync.dma_start(out_d[0:P, :], dbg[:, 0:1024])
                nc.sync.dma_start(out_d[P:2 * P, :], dbg[:, 1024:2048])
                return

        # ================= P3/P4/P5 =================
        with tc.tile_pool(name="ffp" + sfx, bufs=1) as ffp:
            b1f_sb = ffp.tile([P, FT], f32, tag="b1f")
            nc.sync.dma_start(b1f_sb, b1f_d[:, :])
            b2_bc = ffp.tile([P, D], f32, tag="b2")
            nc.sync.dma_start(b2_bc, b2_d.ap().to_broadcast([P, D]))
            h2Tc = [ffp.tile([P, DT, 512], fp8, tag=f"h2T{i}",
                             name=f"h2T{i}")
                    for i in range(2)]
            g1 = ffp.tile([P, FT, 512], fp8, tag="g1")

            def ln2_batch(tiles):
                srcs = [(x2d[qt], bf16) for qt in tiles]
                dsts = [(h2Tc[qt // 4], qt % 4) for qt in tiles]
                ln_batch(srcs, dsts, "l2", ffp)

            def w1_dma(ck, tiles):
                tl = ffp.tile([P, 8, DT, P], fp8, tag="w1r", bufs=2,
                              name="w1r")
                nc.sync.dma_start(
                    tl, w1_d.ap()[ck * 8:(ck + 1) * 8]
                    .rearrange("fc p dt m -> p fc (dt m)"))
                tiles[ck] = tl

            def w1_emit(qc, tiles=None):
                if tiles is None:
                    tiles = {}
                    w1_dma(0, tiles)
                    w1_dma(1, tiles)
                for fc in range(FT):
                    ck = fc // 8
                    if fc % 8 == 0 and ck + 2 <= 3:
                        w1_dma(ck + 2, tiles)
                    tl = tiles[ck]
                    aps = psA.tile([P, 512], f32,
                                   tag=("ctxe" if fc % 2 == 0 else "ctxo"),
                                   name="aps")
                    for dtp in range(4):
                        nc.tensor.matmul(
                            aps,
                            tl[:, fc % 8, 2 * dtp:2 * dtp + 2, :],
                            h2Tc[qc][:, 2 * dtp:2 * dtp + 2, :],
                            start=(dtp == 0), stop=(dtp == 3),
                            perf_mode=DR)
                    nc.scalar.activation(
                        g1[:, fc, :], aps, AF.Gelu,
                        bias=b1f_sb[:, fc:fc + 1], scale=1.0 / W1S)

            def w2_emit(qc, post_mm=None):
                # 8 accumulators (both d-halves x 4 q-tiles) across ALL
                # PSUM banks -> single pass over the ring-loaded W2.
                accs = []
                for j in range(2):
                    sl = psA.tile([P, 1024], f32, tag="sps" + str(j),
                                  name="fps")
                    accs.append(sl[:, 0:512])
                    accs.append(sl[:, 512:1024])
                ce = psA.tile([P, 512], f32, tag="ctxe", name="fpse")
                co = psA.tile([P, 512], f32, tag="ctxo", name="fpso")
                accs += [ce, co, acc_tile(0), acc_tile(1)]
                w2t = {}

                def w2_dma(ck):
                    tl = ffp.tile([P, 4, 2, D], fp8, tag="w2r", bufs=2,
                                  name="w2r")
                    nc.sync.dma_start(
                        tl, w2_d.ap()[ck * 4:(ck + 1) * 4]
                        .rearrange("fc p i n -> p fc (i n)"))
                    w2t[ck] = tl

                w2_dma(0)
                w2_dma(1)
                NF = FT // 2
                for fc in range(NF):
                    ck = fc // 4
                    if fc % 4 == 0 and ck + 2 <= 3:
                        w2_dma(ck + 2)
                    tl = w2t[ck]
                    for j in range(4):
                        for dh in range(2):
                            nc.tensor.matmul(
                                accs[dh * 4 + j],
                                g1[:, 2 * fc:2 * fc + 2,
                                   j * P:(j + 1) * P],
                                tl[:, fc % 4, :,
                                   dh * 512:(dh + 1) * 512],
                                start=(fc == 0), stop=(fc == NF - 1),
                                perf_mode=DR)
                if post_mm is not None:
                    post_mm()
                for dh in range(2):
                    dsl = slice(dh * 512, (dh + 1) * 512)
                    for j in range(4):
                        qt = qc * 4 + j
                        o_t = ffp.tile([P, 512], f32, tag="o_t", bufs=2)
                        x2s = ffp.tile([P, 512], bf16, tag="x2s", bufs=2)
                        nc.sync.dma_start(x2s, x2d[qt, :, dsl])
                        nc.vector.scalar_tensor_tensor(
                            out=o_t, in0=accs[dh * 4 + j],
                            scalar=1.0 / W2S,
                            in1=x2s, op0=ALU.mult, op1=ALU.add)
                        nc.vector.tensor_add(o_t, o_t, b2_bc[:, dsl])
                        nc.sync.dma_start(
                            out_d[qt * P:(qt + 1) * P, dsl], o_t)

            w1t1 = {}
            with tc.tile_pool(name="wop" + sfx, bufs=1) as wop:
                wo_sb = wop.tile([P, HPL, D], bf16, tag="wo")
                nc.sync.dma_start(
                    wo_sb, wo_d.ap().rearrange("(pc p) n -> p pc n", p=P))
                bo_bc = wop.tile([P, D], f32, tag="bo")  # pre-halved
                nc.sync.dma_start(bo_bc, bo_d.ap().to_broadcast([P, D]))

                def wo_tiles(qts):
                    for qt in qts:
                        xo_t = wop.tile([P, D], f32, tag="xo_t", bufs=1)
                        nc.sync.dma_start(
                            xo_t, xl_d[qt * P:(qt + 1) * P, :])
                        x2_t = wop.tile([P, D], bf16, tag="x2_t", bufs=2)
                        for dc in range(2):
                            dsl = slice(dc * 512, (dc + 1) * 512)
                            acc = acc_tile(dc)
                            for pc in range(HPL):
                                nc.tensor.matmul(
                                    acc,
                                    ctxT[:, pc, qt * P:(qt + 1) * P],
                                    wo_sb[:, pc, dsl],
                                    start=(pc == 0),
                                    stop=(pc == HPL - 1))
                            nc.vector.scalar_tensor_tensor(
                                out=x2_t[:, dsl], in0=xo_t[:, dsl],
                                scalar=0.5, in1=acc,
                                op0=ALU.mult, op1=ALU.add)
                            nc.vector.tensor_add(
                                x2_t[:, dsl], x2_t[:, dsl],
                                bo_bc[:, dsl])
                        nc.sync.dma_start(
                            p2d[(qt % 8) // 2][qt // 8, qt % 2], x2_t)

                # P3: Wo chunks 1, 3 -> rs(2), rs(3)
                norm_flush()
                wo_tiles(range(4, 8))
                wo_tiles(range(12, 16))
                rs(2)
                rs(3)

                # P4: attention chunk 2; LN2(4-7) + W1 weight prefetch
                # overlap the ACT-bound attention stretch
                run_chunk(0, 2)
                run_chunk(1, 2)
                norm_flush()
                wo_tiles(range(0, 4))
                run_chunk(2, 2)
                ln2_batch([4, 5])
                run_chunk(3, 2)
                norm_flush()
                ln2_batch([6, 7])
                w1_dma(0, w1t1)
                w1_dma(1, w1t1)
                wo_tiles((8, 9))
                rs(0)
                wo_tiles((10, 11))
                rs(1)

            # P5: FFN qc=1; LN2(0-3) fills the gap while the last W1
            # gelus drain (W2's accumulators WAR-wait them)
            w1_emit(1, w1t1)
            ln2_batch([0, 1, 2, 3])
            w2_emit(1)
            w1_emit(0)
            w2_emit(0)


def _build_program(reps=1):
    nc = bacc.Bacc(None, target_bir_lowering=False)

    xl_d = nc.dram_tensor("xl", (T, D), f32, kind="ExternalInput")
    wq_d = nc.dram_tensor("wq", (D, HL * HD), f32r, kind="ExternalInput")
    wk_d = nc.dram_tensor("wk", (D, HL * HD), f32r, kind="ExternalInput")
    wv_d = nc.dram_tensor("wv", (D, HL * HD), f32r, kind="ExternalInput")
    wo_d = nc.dram_tensor("wo", (HL * HD, D), bf16, kind="ExternalInput")
    # w1: [fc, p, dt, m] pre-arranged fp8 (scaled by W1S)
    w1_d = nc.dram_tensor("w1", (FF // P, P, D // P, P), fp8,
                          kind="ExternalInput")
    # w2: [fc2, p, i, n] pre-arranged fp8 (scaled by W2S), i = k-pair
    w2_d = nc.dram_tensor("w2", (FF // (2 * P), P, 2, D), fp8,
                          kind="ExternalInput")
    qkvb_d = nc.dram_tensor("qkvb", (P, 3 * HPL), f32, kind="ExternalInput")
    lnst_d = nc.dram_tensor("lnst", (P, NT, 2), f32, kind="ExternalInput")
    bo_d = nc.dram_tensor("bo_", (1, D), f32, kind="ExternalInput")
    b1f_d = nc.dram_tensor("b1f", (P, FF // P), f32, kind="ExternalInput")
    b2_d = nc.dram_tensor("b2_", (1, D), f32, kind="ExternalInput")
    out_d = nc.dram_tensor("out", (TQ, D), f32, kind="ExternalOutput")

    with tile.TileContext(nc) as tc:
        with (
            tc.tile_pool(name="const", bufs=1) as const,
            tc.tile_pool(name="dramp", bufs=1, space="DRAM") as dramp,
        ):
            ident_f = const.tile([P, P], f32)
            make_identity(nc, ident_f)
            ident = const.tile([P, P], f32r)
            nc.vector.tensor_copy(ident, ident_f)
            ident_b = const.tile([P, P], bf16)
            nc.vector.tensor_copy(ident_b, ident_f)
            # S^T-space causal keep mask: keep where kv(part) <= q(free)
            tril_f = const.tile([P, P], f32)
            nc.gpsimd.memset(tril_f, 1.0)
            nc.gpsimd.affine_select(
                out=tril_f, in_=tril_f, compare_op=ALU.is_ge, fill=0.0,
                base=0, pattern=[[1, P]], channel_multiplier=-1,
            )
            tril = const.tile([P, P], f32r)
            nc.vector.tensor_copy(tril, tril_f)
            ones16 = const.tile([P, NT], f32)
            nc.vector.memset(ones16, 1.0)
            qkvb = const.tile([P, 3 * HPL], f32)
            nc.sync.dma_start(qkvb, qkvb_d[:, :])
            x2d = dramp.tile([NQ, P, D], bf16)  # post-RS own-half residual
            # partial attn out (bf16), one contiguous [half, tile] buffer
            # per ReduceScatter chunk of two row-tiles
            p2d = []
            for k in range(4):
                p2_k = dramp.tile([2, 2, P, D], bf16, tag=f"p2{k}")
                p2d.append(p2_k)

            cst = (ident, ident_b, tril, ones16, qkvb)
            dram = (xl_d, wq_d, wk_d, wv_d, wo_d, w1_d, w2_d,
                    bo_d, b1f_d, b2_d, out_d, lnst_d)
            for rep in range(reps):
                sfx = f"r{rep}" if reps > 1 else ""
                _emit_body(nc, tc, sfx, cst, x2d, p2d, dram)

    nc.compile()
    return nc


def _prep_inputs(inputs):
    """Host-side: fold LN affine + score scale into weights; build per-core maps."""
    x = np.asarray(inputs["x"], dtype=np.float32)
    g1, b1_ = np.asarray(inputs["ln1_g"], np.float32), np.asarray(inputs["ln1_b"], np.float32)
    g2, b2_ = np.asarray(inputs["ln2_g"], np.float32), np.asarray(inputs["ln2_b"], np.float32)
    Wq = np.asarray(inputs["Wq"], np.float32)  # [H, D, HD]
    Wk = np.asarray(inputs["Wk"], np.float32)
    Wv = np.asarray(inputs["Wv"], np.float32)
    bq = np.asarray(inputs["bq"], np.float32)  # [H, HD]
    bk = np.asarray(inputs["bk"], np.float32)
    bv = np.asarray(inputs["bv"], np.float32)
    Wo = np.asarray(inputs["Wo"], np.float32)
    bo = np.asarray(inputs["bo"], np.float32)
    W1 = np.asarray(inputs["W1"], np.float32)
    b1 = np.asarray(inputs["b1"], np.float32)
    W2 = np.asarray(inputs["W2"], np.float32)
    b2 = np.asarray(inputs["b2"], np.float32)

    sc = 1.0 / np.sqrt(np.float32(HD))
    # [H, D, HD] -> [D, H*HD]
    wq_flat = np.transpose(Wq, (1, 0, 2)).reshape(D, D)
    wk_flat = np.transpose(Wk, (1, 0, 2)).reshape(D, D)
    wv_flat = np.transpose(Wv, (1, 0, 2)).reshape(D, D)
    wq_f = (g1[:, None] * wq_flat) * sc
    wk_f = g1[:, None] * wk_flat
    wv_f = g1[:, None] * wv_flat
    bq_f = (b1_ @ wq_flat + bq.reshape(D)) * sc
    bk_f = b1_ @ wk_flat + bk.reshape(D)
    bv_f = b1_ @ wv_flat + bv.reshape(D)

    w1_f = g2[:, None] * W1
    b1_f = (b2_ @ W1 + b1).reshape(FF // P, P).T.copy()  # [P, FF//P]

    import ml_dtypes

    e4m3 = ml_dtypes.float8_e4m3fn
    DT_, FT_ = D // P, FF // P
    # [fc, p, dt, m] layout; scaled into fp8's normal range
    w1_8 = np.clip(w1_f * W1S, -240, 240).astype(e4m3)
    w1_8 = w1_8.reshape(DT_, P, FT_, P).transpose(2, 1, 0, 3).copy()
    w2_8 = np.clip(W2 * W2S, -240, 240).astype(e4m3)
    w2_8 = w2_8.reshape(FT_ // 2, 2, P, D).transpose(0, 2, 1, 3).copy()

    shared = {
        "w1": w1_8, "w2": w2_8,
        "bo_": np.ascontiguousarray(0.5 * bo.reshape(1, D)),
        "b1f": np.ascontiguousarray(b1_f), "b2_": b2.reshape(1, D),
    }
    # per head-half: weight column/row slices + bias table
    half = {}
    for o in range(2):
        hsl = slice(o * HL * HD, (o + 1) * HL * HD)
        qkvb = np.zeros((P, 3 * HPL), np.float32)
        for hp in range(HPL):
            gp = o * HPL + hp  # global head pair
            for wi, bf in enumerate((bq_f, bk_f, bv_f)):
                qkvb[0:HD, 3 * hp + wi] = bf[(2 * gp) * HD:(2 * gp + 1) * HD]
                qkvb[HD:P, 3 * hp + wi] = bf[(2 * gp + 1) * HD:(2 * gp + 2) * HD]
        half[o] = {
            "wq": np.ascontiguousarray(wq_f[:, hsl]),
            "wk": np.ascontiguousarray(wk_f[:, hsl]),
            "wv": np.ascontiguousarray(wv_f[:, hsl]),
            "wo": np.ascontiguousarray(Wo[hsl, :]).astype(ml_dtypes.bfloat16),
            "qkvb": qkvb,
        }
    lnsts = []
    for b in range(B):
        xb = x[b].astype(np.float64)
        mu = xb.mean(1)
        rstd = 1.0 / np.sqrt(xb.var(1) + 1e-5)
        lt = np.empty((P, T // P, 2), np.float32)
        lt[:, :, 0] = (-mu * rstd).reshape(T // P, P).T
        lt[:, :, 1] = rstd.reshape(T // P, P).T
        lnsts.append(lt)
    in_maps = []
    for c in range(8):
        b, o = c // 2, c % 2
        m = dict(shared)
        m.update(half[o])
        m["xl"] = np.ascontiguousarray(x[b])
        m["lnst"] = lnsts[b]
        in_maps.append(m)
    return in_maps


def kernel(**inputs):
    if "nc" not in _CACHE:
        _CACHE["nc"] = _build_program()
    nc = _CACHE["nc"]
    in_maps = _prep_inputs(inputs)
    res = run_bass_kernel_spmd(nc, in_maps, core_ids=list(range(8)))
    out = np.empty((B, T, D), np.float32)
    for c in range(8):
        b, o = c // 2, c % 2
        out[b, o * TQ:(o + 1) * TQ] = res.results[c]["out"]
    return out
